# revision 45
# baseline (speedup 1.0000x reference)
"""CPRLinear Trainium2 kernel.

y = x[:, col_indices] @ W_deq.T + bias, where W_deq is the per-128-column-tile
affine dequantization of [W_high_q | W_low_q] (int codes, values 0..63).

Sharding: out_features (8192) split across 8 NeuronCores, 1024 rows each.
x / col_indices replicated (x is shipped transposed bf16 so the column
permutation becomes a contiguous row gather on device).

Changes vs the int32 baseline (~212-254 us; this kernel 150.2 us):
  - codes ship as ONE int8 tensor [O_SLAB, IN] (lossless: values < 64);
    weight HBM+SBUF traffic drops 4x (33.5 -> 8.4 MB/core)
  - x ships bf16; its permutation runs as 8 batched SWDGE dma_gathers
    (<=1024 rows/call - 2048 overflows the SWDGE descriptor ring and
    crashes HW), ALL issued upfront into a full 4.2MB SBUF stage so the
    Pool engine never paces the chunk pipeline
  - dequant uses the scale-only decomposition
        y = x_perm @ (q*s).T - S1 @ (z*s).T + bias,   S1[t,b] = sum_j x[t,j,b]
    so the per-128-col-tile affine collapses to ONE broadcast
    tensor_tensor (q * s, stride-0 inner axis on the scale) per
    (o-block, chunk): 40 DVE instructions instead of 512 per-tile
    tensor_scalars (~270ns fixed overhead each; DVE dequant 191us -> 75us
    measured). S1 accumulates on TensorE via one-hot stationaries (one
    N=256 matmul per k-tile into a [64,256] PSUM group) and the zero-point
    term lands as 4 small matmuls before the bias matmul closes each group.
    GpSimd tensor ops measured ~5x slower - never offload dequant there
  - transposes ride the Sync ring ALONE (each DMA_TRANSPOSE occupies its
    issuing queue for the transfer); weight/scale loads ride the ACT ring
  - 8 uniform 1024-k chunks with a DEEP load lead (wq pool = 5 chunks):
    head-of-line waits on the strict-FIFO ACT load queue propagate
    just-in-time-ness down the whole load->dequant->transpose->MM chain,
    stalling the PE ~6.5us every chunk (HAM re-throttles each time);
    wq depth 1/3/4/5 chunks measured 204/173/170/150us. The S1 matmuls
    sit between the oc halves to bridge the h1-transpose wait
  - x staged in 8 SEPARATE tiles: slice-writes into one shared tile get
    serialized by Tile with ~7.5us gpsimd DRAINs between gathers

Per-core device pipeline:
  - x: 8 upfront dma_gathers of bf16 xT rows by col_indices (int16
    wrapped 16-partition index layout) -> xall [128k, kt, 256b]
  - W (per chunk): HWDGE int8 loads (natural [o,k] row slabs), one
    broadcast-TT q*s per o-block into per-half staging [128o, (t, obh, k)],
    then ONE xbar DMA-transpose per half (both halves SAME ring;
    cross-ring split corrupted on HW). Transposed 128x128 blocks land at
    wt[k, oc, t, obh, o] so each matmul rhs is one contiguous 512-run
  - TensorE: y[b,o] over 64 k-tiles in 4 PSUM groups (2 b-blocks x 2
    o-halves, N=512, oc-major so 16-MM runs stay on one PSUM group) plus
    the S1 group; epilogue adds the zero-point correction and bias
  - DVE evacuates PSUM -> SBUF, HWDGE stores y [256, 1024] f32; host
    concatenates slabs along out_features
"""

import os
import sys

import numpy as np

for _p in ("/root/.axon_site", "/root/.axon_site/_ro/trn_rl_repo",
           "/root/.axon_site/_ro/pypackages", "/opt/trn_rl_repo"):
    if os.path.isdir(_p) and _p not in sys.path:
        sys.path.append(_p)

B, IN, OUT = 256, 8192, 8192
N_CORES = 8
O_SLAB = OUT // N_CORES          # 1024 out rows per core
N_HIGH, N_LOW = 2048, 6144
TILE = 128
NT = IN // TILE                  # 64 k-tiles
OB = O_SLAB // TILE              # 8 o-blocks per core
# (k_offset, k_len) chunks; small leading chunks shorten the ramp to the
# first matmul. Codes are one merged tensor so chunks may straddle the
# high/low boundary freely.
CHUNK_PLAN = [(k, 1024) for k in range(0, IN, 1024)]
# of every 16 dequant tiles, DVE_W go to DVE, 16-DVE_W to ACT
# (HW: DVE tensor_scalar ~540-630ns/instr contended, ACT stable ~480ns)
DVE_W = 11

_PROGRAM = None


def _build_program(n_bodies=1):
    import concourse.bass as bass
    import concourse.bacc as bacc
    import concourse.tile as tile
    import concourse.mybir as mybir

    f32 = mybir.dt.float32
    i32 = mybir.dt.int32
    i8 = mybir.dt.int8
    bf16 = mybir.dt.bfloat16

    nc = bacc.Bacc(
        "TRN2",
        target_bir_lowering=False,
        debug=False,
        enable_asserts=False,
        num_devices=N_CORES,
    )

    xT = nc.dram_tensor("xT", [IN, B], bf16, kind="ExternalInput").ap()
    ci = nc.dram_tensor("ci", [128, NT], i32, kind="ExternalInput").ap()
    # col_indices int16, 16-partition-wrapped + replicated (dma_gather layout)
    cig = nc.dram_tensor("cig", [128, IN // 16], mybir.dt.int16,
                         kind="ExternalInput").ap()
    wq = nc.dram_tensor("wq", [O_SLAB, IN], i8, kind="ExternalInput").ap()
    sT = nc.dram_tensor("sT", [OB, 128, NT], f32, kind="ExternalInput").ap()
    snat = nc.dram_tensor("snat", [NT, O_SLAB], f32, kind="ExternalInput").ap()
    znat = nc.dram_tensor("znat", [NT, O_SLAB], f32, kind="ExternalInput").ap()
    eoh = nc.dram_tensor("eoh", [128, NT * 64], mybir.dt.bfloat16,
                         kind="ExternalInput").ap()
    bias = nc.dram_tensor("bias", [1, O_SLAB], f32, kind="ExternalInput").ap()
    y = nc.dram_tensor("y", [B, O_SLAB], f32, kind="ExternalOutput").ap()

    with tile.TileContext(nc) as tc:
        for _ in range(n_bodies):
            _kernel_body(tc, xT, ci, cig, wq, sT, snat, znat, eoh, bias, y,
                         bass=bass, mybir=mybir, tile=tile)

    nc.compile()
    return nc


def _kernel_body(tc, xT, ci, cig, wq, sT, snat, znat, eoh, bias, y, *,
                 bass, mybir, tile):
    from contextlib import ExitStack

    nc = tc.nc
    f32 = mybir.dt.float32
    bf16 = mybir.dt.bfloat16
    i32 = mybir.dt.int32
    i8 = mybir.dt.int8
    Alu = mybir.AluOpType
    Act = mybir.ActivationFunctionType

    with ExitStack() as ctx:
        const = ctx.enter_context(tc.tile_pool(name="const", bufs=1))
        xstage = ctx.enter_context(tc.tile_pool(name="xstage", bufs=1))
        wqpool = ctx.enter_context(tc.tile_pool(name="wq", bufs=40))
        wnpool = ctx.enter_context(tc.tile_pool(name="wn", bufs=6))
        wtpool = ctx.enter_context(tc.tile_pool(name="wt", bufs=3))
        ypool = ctx.enter_context(tc.tile_pool(name="yout", bufs=2))
        psum = ctx.enter_context(tc.tile_pool(name="psum", bufs=1, space="PSUM"))

        # --- constants ---
        old_gather = bool(os.environ.get("KERNEL_OLD_GATHER"))
        if old_gather:
            ci_sb = const.tile([128, NT], i32, tag="ci")
            nc.sync.dma_start(out=ci_sb, in_=ci)
        else:
            cig_sb = const.tile([128, IN // 16], mybir.dt.int16, tag="cig")
            nc.scalar.dma_start(out=cig_sb, in_=cig)

        # scales, o-partition-major flat [128, OB*NT]: broadcast operand
        # of the q*s dequant multiply
        sT_sb = const.tile([128, OB * NT], f32, tag="sT")
        for ob in range(OB):
            nc.scalar.dma_start(out=sT_sb[:, ob * NT:(ob + 1) * NT], in_=sT[ob])

        # t-major scale/zero tables for the zero-point correction matmul:
        # nzs[t, o] = -(z * s), bf16
        snat_sb = const.tile([64, O_SLAB], f32, tag="snat")
        znat_sb = const.tile([64, O_SLAB], f32, tag="znat")
        nc.scalar.dma_start(out=snat_sb, in_=snat)
        nc.scalar.dma_start(out=znat_sb, in_=znat)
        nzs_f = const.tile([64, O_SLAB], f32, tag="nzsf")
        nc.vector.tensor_tensor(
            out=nzs_f[:, :], in0=znat_sb[:, :], in1=snat_sb[:, :], op=Alu.mult)
        nzs_b = const.tile([64, O_SLAB], bf16, tag="nzsb")
        nc.vector.tensor_scalar(
            out=nzs_b[:, :], in0=nzs_f[:, :],
            scalar1=-1.0, scalar2=None, op0=Alu.mult)

        # one-hot stationaries for the per-tile x column-sum (S1) matmuls
        eoh_sb = const.tile([128, NT * 64], bf16, tag="eoh")
        nc.scalar.dma_start(out=eoh_sb, in_=eoh)

        ones = const.tile([128, 128], bf16, tag="ones")
        nc.vector.memset(ones, 1.0)

        # HAM warm-up: the ~30us matmul-free W-chain ramp leaves the PE
        # clock at 1.2GHz (K=4/8) when the real stream starts (~425ns/MM
        # measured vs ~214 warm). Fill the ramp with one ACCUMULATING
        # dummy matmul group (same pattern as the real y-groups; 110
        # independent start/stop groups crashed the exec unit) and
        # evacuate it like a normal group.
        wup = const.tile([128, 512], bf16, tag="wup")
        nc.vector.memset(wup, 0.0)
        psd = psum.tile([128, 512], f32, tag="psd", name="psd")
        for i in range(110):
            nc.tensor.matmul(psd[:, :], ones, wup[:, :],
                             start=(i == 0), stop=(i == 109))
        wup_rd = const.tile([128, 512], f32, tag="wupr")
        nc.vector.tensor_copy(wup_rd, psd[:, :])

        wbias = const.tile([128, O_SLAB], bf16, tag="wbias")
        nc.vector.memset(wbias, 0.0)
        bias_f = const.tile([1, O_SLAB], f32, tag="biasf")
        nc.scalar.dma_start(out=bias_f, in_=bias)
        nc.vector.tensor_copy(wbias[0:1, :], bias_f)

        # ---- x path: stage ALL of permuted x upfront (4.2 MB, 32KB/
        # partition). The gathers depend only on xT+indices, so the Pool
        # engine emits descriptors flat-out from t=0 instead of pacing the
        # per-chunk pipeline.
        # one tile per 1024-k block: writes to a single shared tile get
        # serialized by Tile with ~7.5us gpsimd DRAINs between gathers,
        # stretching the x feed to ~170us and starving the mid-game MMs
        xtiles = [xstage.tile([128, 8, B], bf16, tag=f"xall{g}",
                              name=f"xall{g}") for g in range(NT // 8)]
        if old_gather:
            for kt in range(NT):
                nc.gpsimd.indirect_dma_start(
                    out=xtiles[kt // 8][:, kt % 8, :],
                    out_offset=None,
                    in_=xT,
                    in_offset=bass.IndirectOffsetOnAxis(
                        ap=ci_sb[:, kt:kt + 1], axis=0),
                )
        else:
            for g in range(IN // 1024):
                # >=2048 rows per gather overflows the SWDGE descriptor
                # ring (crashes HW); 1024 is the safe max
                nc.gpsimd.dma_gather(
                    xtiles[g][:, :, :],
                    xT,
                    cig_sb[:, g * 64:(g + 1) * 64],
                    1024,
                    1024,
                    B,
                )

        # PSUM accumulation groups: [b-block][o-half]
        ps = [[psum.tile([128, 512], f32, tag=f"ps{bb}{oc}", name=f"ps{bb}{oc}")
               for oc in range(2)] for bb in range(2)]
        # per-k-tile x column sums S1[t, b], one long accumulation group
        s1ps = psum.tile([64, B], f32, tag="s1ps", name="s1ps")

        for ci_, (k_off, k_len) in enumerate(CHUNK_PLAN):
            tpc = k_len // 128
            # ---- W path: load int8, dequant, transpose to k-major ----
            # wt layout: [k-in-tile 128, half, t, ob-in-half, o-in-block 128]
            wt = wtpool.tile([128, 2, tpc, OB // 2, 128], bf16, tag="wt",
                             name=f"wt{ci_}")
            # per-half dequant staging [o-in-block, (t, obh, k)] so one xbar
            # transpose instruction covers many 128x128 blocks, landing at
            # wt[:, h, t, obh, :] directly
            wnh = [wnpool.tile([128, tpc, OB // 2, 128], bf16, tag="wn",
                               name=f"wn{ci_}h{h}") for h in range(2)]
            for ob in range(OB):
                wn, obh = wnh[ob // (OB // 2)], ob % (OB // 2)
                wq_sb = wqpool.tile([128, tpc, 128], i8, tag="wq",
                                    name=f"wq{ci_}o{ob}")
                nc.scalar.dma_start(
                    out=wq_sb,
                    in_=wq[ob * 128:(ob + 1) * 128,
                           k_off:k_off + k_len].rearrange(
                               "p (t k) -> p t k", t=tpc))
                # scale-only dequant q*s as ONE broadcast tensor_tensor per
                # (ob, chunk): the per-tile scale rides a stride-0 inner
                # axis, cutting 512 per-tile instructions (~270ns fixed
                # overhead each) to 40. The zero-point term is restored
                # exactly by the S1 correction matmuls below.
                sc0 = ob * NT + k_off // 128
                nc.vector.tensor_tensor(
                    out=wn[:, :, obh, :],
                    in0=wq_sb[:, :, :],
                    in1=sT_sb[:, sc0:sc0 + tpc, None].to_broadcast(
                        [128, tpc, 128]),
                    op=Alu.mult,
                )
            # both halves on the ACT ring (same-ring transposes are safe;
            # cross-ring split corrupted on HW) - oc=0 MMs can start after
            # the first half lands
            # both halves on the SAME ring (cross-ring split corrupted on
            # HW); Sync ring is otherwise transfer-free so the ~8.5us-per-
            # transpose queue occupancy does not block loads or dequant
            nc.sync.dma_start_transpose(
                wt[:, 0, :, :, :],
                wnh[0][:, :, :, :].rearrange("p a b c -> p (a b c)"))
            nc.sync.dma_start_transpose(
                wt[:, 1, :, :, :],
                wnh[1][:, :, :, :].rearrange("p a b c -> p (a b c)"))

            # ---- matmuls: accumulate y over this chunk's k-tiles ----
            # oc-major: the oc half only depends on its half-transpose.
            # The S1 matmuls (which depend only on xall) sit BETWEEN the
            # halves: they bridge the wait for the h1 transpose and keep
            # the PE HAM clock warm
            for oc in range(2):
                if oc == 1:
                    for t in range(tpc):
                        kt = k_off // 128 + t
                        nc.tensor.matmul(
                            s1ps[:, :],
                            eoh_sb[:, kt * 64:(kt + 1) * 64],
                            xtiles[kt // 8][:, kt % 8, :],
                            start=(kt == 0),
                            stop=(kt == NT - 1),
                        )
                for t in range(tpc):
                    kt = k_off // 128 + t
                    for bb in range(2):
                        nc.tensor.matmul(
                            ps[bb][oc][:, :],
                            xtiles[kt // 8][:, kt % 8,
                                            bb * 128:(bb + 1) * 128],
                            wt[:, oc, t, :, :],
                            start=(kt == 0),
                            stop=False,
                        )

        # ---- epilogue: zero-point correction + bias close each group ----
        # y -= S1 @ (z*s): lhsT = S1[t, b] bf16, rhs = -(z*s)[t, o] bf16,
        # contraction over the 64 k-tiles
        s1bf = const.tile([64, B], bf16, tag="s1bf")
        nc.scalar.copy(s1bf, s1ps[:, :])
        for oc in range(2):
            for bb in range(2):
                nc.tensor.matmul(
                    ps[bb][oc][:, :],
                    s1bf[:, bb * 128:(bb + 1) * 128],
                    nzs_b[:, oc * 512:(oc + 1) * 512],
                    start=False,
                    stop=False,
                )
                nc.tensor.matmul(
                    ps[bb][oc][:, :],
                    ones,
                    wbias[:, oc * 512:(oc + 1) * 512],
                    start=False,
                    stop=True,
                )
                ysb = ypool.tile([128, 512], f32, tag="ysb")
                nc.vector.tensor_copy(ysb, ps[bb][oc][:, :])
                nc.sync.dma_start(
                    out=y[bb * 128:(bb + 1) * 128, oc * 512:(oc + 1) * 512],
                    in_=ysb,
                )


def get_program():
    global _PROGRAM
    if _PROGRAM is None:
        _PROGRAM = _build_program()
    return _PROGRAM


def make_in_maps(x, W_high_q, W_low_q, scales_high, zeros_high,
                 scales_low, zeros_low, bias, col_indices):
    """Host-side sharding / layout prep. Returns per-core input dicts."""
    import concourse.mybir as mybir
    bf16 = mybir.dt.np(mybir.dt.bfloat16)

    x = np.asarray(x)
    xT = np.ascontiguousarray(x.T.astype(np.float32, copy=False).astype(bf16))
    ci = np.ascontiguousarray(
        np.asarray(col_indices).astype(np.int32, copy=False).reshape(NT, 128).T
    )  # [128, NT]; ci[p, t] = col_indices[t*128 + p]
    # dma_gather index layout: int16, wrapped into 16 partitions
    # (idx j at [j%16, j//16]), replicated to all 128 partitions
    cig = np.tile(
        np.asarray(col_indices).astype(np.int16, copy=False).reshape(IN // 16, 16).T,
        (8, 1),
    )  # [128, IN//16]

    s_all = np.concatenate(
        [np.asarray(scales_high, dtype=np.float32),
         np.asarray(scales_low, dtype=np.float32)], axis=0)   # [NT, OUT]
    z_all = np.concatenate(
        [np.asarray(zeros_high, dtype=np.float32),
         np.asarray(zeros_low, dtype=np.float32)], axis=0)    # [NT, OUT]
    sT_full = np.ascontiguousarray(s_all.T)                   # [OUT, NT]
    zT_full = np.ascontiguousarray(z_all.T)                   # [OUT, NT]

    # codes are 0..63: pack losslessly to int8, one merged [OUT, IN] tensor
    wq_full = np.concatenate(
        [np.asarray(W_high_q), np.asarray(W_low_q)], axis=1).astype(np.int8)
    bias = np.asarray(bias, dtype=np.float32)

    # one-hot stationaries for the S1 (per-tile x column sum) matmuls:
    # eoh[:, kt*64 + m] = 1 iff m == kt
    eoh = np.zeros((128, NT, 64), dtype=np.float32)
    for kt in range(NT):
        eoh[:, kt, kt] = 1.0
    eoh = np.ascontiguousarray(eoh.reshape(128, NT * 64).astype(bf16))

    in_maps = []
    for c in range(N_CORES):
        sl = slice(c * O_SLAB, (c + 1) * O_SLAB)
        in_maps.append({
            "xT": xT,
            "ci": ci,
            "cig": np.ascontiguousarray(cig),
            "wq": np.ascontiguousarray(wq_full[sl]),
            "sT": np.ascontiguousarray(sT_full[sl].reshape(OB, 128, NT)),
            "snat": np.ascontiguousarray(s_all[:, sl]),
            "znat": np.ascontiguousarray(z_all[:, sl]),
            "eoh": eoh,
            "bias": np.ascontiguousarray(bias[sl].reshape(1, O_SLAB)),
        })
    return in_maps


def run_on_device(in_maps):
    from concourse.bass_utils import run_bass_kernel_spmd
    nc = get_program()
    res = run_bass_kernel_spmd(nc, in_maps, list(range(N_CORES)))
    out = np.concatenate(
        [res.results[c]["y"] for c in range(N_CORES)], axis=1)
    return np.ascontiguousarray(out.astype(np.float32, copy=False))


def kernel(x, W_high_q, W_low_q, scales_high, zeros_high,
           scales_low, zeros_low, bias, col_indices):
    in_maps = make_in_maps(x, W_high_q, W_low_q, scales_high, zeros_high,
                           scales_low, zeros_low, bias, col_indices)
    return run_on_device(in_maps)


# ---------------------------------------------------------------------------
# Benchmark path (test.py only): inputs parked on-device, jit built once,
# dispatches pipelined so the axon-tunnel round trip amortizes away.
# ---------------------------------------------------------------------------

class DeviceRunner:
    def __init__(self, in_maps, nc=None):
        import jax
        import numpy as _np
        from jax.experimental.shard_map import shard_map
        from jax.sharding import Mesh, NamedSharding, PartitionSpec
        import concourse.mybir as mybir
        from concourse.bass2jax import (
            _bass_exec_p, install_neuronx_cc_hook, partition_id_tensor)

        install_neuronx_cc_hook()
        if nc is None:
            nc = get_program()
        partition_name = (nc.partition_id_tensor.name
                          if nc.partition_id_tensor else None)

        in_names, out_names, out_avals, zero_outs = [], [], [], []
        for alloc in nc.m.functions[0].allocations:
            if not isinstance(alloc, mybir.MemoryLocationSet):
                continue
            name = alloc.memorylocations[0].name
            if alloc.kind == "ExternalInput":
                if name != partition_name:
                    in_names.append(name)
            elif alloc.kind == "ExternalOutput":
                shape = tuple(alloc.tensor_shape)
                dtype = mybir.dt.np(alloc.dtype)
                out_names.append(name)
                out_avals.append(jax.core.ShapedArray(shape, dtype))
                zero_outs.append(_np.zeros(shape, dtype))
        n_params = len(in_names)
        all_in_names = list(in_names) + list(out_names)
        if partition_name is not None:
            all_in_names.append(partition_name)

        def _body(*args):
            operands = list(args)
            if partition_name is not None:
                operands.append(partition_id_tensor())
            return tuple(_bass_exec_p.bind(
                *operands,
                out_avals=tuple(out_avals),
                in_names=tuple(all_in_names),
                out_names=tuple(out_names),
                lowering_input_output_aliases=(),
                sim_require_finite=True,
                sim_require_nnan=True,
                nc=nc,
            ))

        devices = jax.devices()[:N_CORES]
        mesh = Mesh(_np.asarray(devices), ("core",))
        spec = PartitionSpec("core")
        nin = n_params + len(zero_outs)
        self.fn = jax.jit(
            shard_map(_body, mesh=mesh,
                      in_specs=(spec,) * nin,
                      out_specs=(spec,) * len(out_names),
                      check_rep=False),
            keep_unused=True,
        )
        sharding = NamedSharding(mesh, spec)
        concat_in = [
            _np.concatenate([in_maps[c][k] for c in range(N_CORES)], axis=0)
            for k in in_names
        ]
        concat_zeros = [
            _np.zeros((N_CORES * z.shape[0], *z.shape[1:]), z.dtype)
            for z in zero_outs
        ]
        self.args = [jax.device_put(a, sharding)
                     for a in concat_in + concat_zeros]
        self.out_names = out_names
        self.out_avals = out_avals
        self._jax = jax

    def run(self):
        return self.fn(*self.args)

    def fetch(self, outs):
        import numpy as _np
        y = _np.asarray(outs[self.out_names.index("y")])
        y = y.reshape(N_CORES, B, O_SLAB)
        return _np.concatenate(list(y), axis=1)

    def bench(self, iters=20):
        import time
        jax = self._jax
        # warm
        outs = self.run()
        jax.block_until_ready(outs)
        t0 = time.perf_counter()
        last = None
        for _ in range(iters):
            last = self.run()
        jax.block_until_ready(last)
        dt = (time.perf_counter() - t0) / iters
        return dt, self.fetch(last)


# revision 46
# speedup vs baseline: 1.4192x; 1.4192x over previous
"""CPRLinear Trainium2 kernel.

y = x[:, col_indices] @ W_deq.T + bias, where W_deq is the per-128-column-tile
affine dequantization of [W_high_q | W_low_q] (int codes, values 0..63).

Sharding: out_features (8192) split across 8 NeuronCores, 1024 rows each.
x / col_indices replicated (x is shipped transposed bf16 so the column
permutation becomes a contiguous row gather on device).

Changes vs the int32 baseline (~212-254 us; this kernel 150.2 us):
  - codes ship as ONE int8 tensor [O_SLAB, IN] (lossless: values < 64);
    weight HBM+SBUF traffic drops 4x (33.5 -> 8.4 MB/core)
  - x ships bf16; its permutation runs as 8 batched SWDGE dma_gathers
    (<=1024 rows/call - 2048 overflows the SWDGE descriptor ring and
    crashes HW), ALL issued upfront into a full 4.2MB SBUF stage so the
    Pool engine never paces the chunk pipeline
  - dequant uses the scale-only decomposition
        y = x_perm @ (q*s).T - S1 @ (z*s).T + bias,   S1[t,b] = sum_j x[t,j,b]
    so the per-128-col-tile affine collapses to ONE broadcast
    tensor_tensor (q * s, stride-0 inner axis on the scale) per
    (o-block, chunk): 40 DVE instructions instead of 512 per-tile
    tensor_scalars (~270ns fixed overhead each; DVE dequant 191us -> 75us
    measured). S1 accumulates on TensorE via one-hot stationaries (one
    N=256 matmul per k-tile into a [64,256] PSUM group) and the zero-point
    term lands as 4 small matmuls before the bias matmul closes each group.
    GpSimd tensor ops measured ~5x slower - never offload dequant there
  - transposes ride the Sync ring ALONE (each DMA_TRANSPOSE occupies its
    issuing queue for the transfer); weight/scale loads ride the ACT ring
  - 8 uniform 1024-k chunks with a DEEP load lead (wq pool = 5 chunks):
    head-of-line waits on the strict-FIFO ACT load queue propagate
    just-in-time-ness down the whole load->dequant->transpose->MM chain,
    stalling the PE ~6.5us every chunk (HAM re-throttles each time);
    wq depth 1/3/4/5 chunks measured 204/173/170/150us. The S1 matmuls
    sit between the oc halves to bridge the h1-transpose wait
  - x staged in 8 SEPARATE tiles: slice-writes into one shared tile get
    serialized by Tile with ~7.5us gpsimd DRAINs between gathers

Per-core device pipeline:
  - x: 8 upfront dma_gathers of bf16 xT rows by col_indices (int16
    wrapped 16-partition index layout) -> xall [128k, kt, 256b]
  - W (per chunk): HWDGE int8 loads (natural [o,k] row slabs), one
    broadcast-TT q*s per o-block into per-half staging [128o, (t, obh, k)],
    then ONE xbar DMA-transpose per half (both halves SAME ring;
    cross-ring split corrupted on HW). Transposed 128x128 blocks land at
    wt[k, oc, t, obh, o] so each matmul rhs is one contiguous 512-run
  - TensorE: y[b,o] over 64 k-tiles in 4 PSUM groups (2 b-blocks x 2
    o-halves, N=512, oc-major so 16-MM runs stay on one PSUM group) plus
    the S1 group; epilogue adds the zero-point correction and bias
  - DVE evacuates PSUM -> SBUF, HWDGE stores y [256, 1024] f32; host
    concatenates slabs along out_features
"""

import os
import sys

import numpy as np

for _p in ("/root/.axon_site", "/root/.axon_site/_ro/trn_rl_repo",
           "/root/.axon_site/_ro/pypackages", "/opt/trn_rl_repo"):
    if os.path.isdir(_p) and _p not in sys.path:
        sys.path.append(_p)

B, IN, OUT = 256, 8192, 8192
N_CORES = 8
O_SLAB = OUT // N_CORES          # 1024 out rows per core
N_HIGH, N_LOW = 2048, 6144
TILE = 128
NT = IN // TILE                  # 64 k-tiles
OB = O_SLAB // TILE              # 8 o-blocks per core
# (k_offset, k_len) chunks; small leading chunks shorten the ramp to the
# first matmul. Codes are one merged tensor so chunks may straddle the
# high/low boundary freely.
CHUNK_PLAN = [(k, 1024) for k in range(0, IN, 1024)]
# of every 16 dequant tiles, DVE_W go to DVE, 16-DVE_W to ACT
# (HW: DVE tensor_scalar ~540-630ns/instr contended, ACT stable ~480ns)
DVE_W = 11

_PROGRAM = None


def _build_program(n_bodies=1):
    import concourse.bass as bass
    import concourse.bacc as bacc
    import concourse.tile as tile
    import concourse.mybir as mybir

    f32 = mybir.dt.float32
    i32 = mybir.dt.int32
    i8 = mybir.dt.int8
    bf16 = mybir.dt.bfloat16

    nc = bacc.Bacc(
        "TRN2",
        target_bir_lowering=False,
        debug=False,
        enable_asserts=False,
        num_devices=N_CORES,
    )

    xT = nc.dram_tensor("xT", [IN, B], bf16, kind="ExternalInput").ap()
    ci = nc.dram_tensor("ci", [128, NT], i32, kind="ExternalInput").ap()
    # col_indices int16, 16-partition-wrapped + replicated (dma_gather layout)
    cig = nc.dram_tensor("cig", [128, IN // 16], mybir.dt.int16,
                         kind="ExternalInput").ap()
    wq = nc.dram_tensor("wq", [O_SLAB, IN], i8, kind="ExternalInput").ap()
    sT = nc.dram_tensor("sT", [OB, 128, NT], f32, kind="ExternalInput").ap()
    snat = nc.dram_tensor("snat", [NT, O_SLAB], f32, kind="ExternalInput").ap()
    znat = nc.dram_tensor("znat", [NT, O_SLAB], f32, kind="ExternalInput").ap()
    eoh = nc.dram_tensor("eoh", [128, NT * 64], mybir.dt.bfloat16,
                         kind="ExternalInput").ap()
    bias = nc.dram_tensor("bias", [1, O_SLAB], f32, kind="ExternalInput").ap()
    y = nc.dram_tensor("y", [B, O_SLAB], f32, kind="ExternalOutput").ap()

    with tile.TileContext(nc) as tc:
        for _ in range(n_bodies):
            _kernel_body(tc, xT, ci, cig, wq, sT, snat, znat, eoh, bias, y,
                         bass=bass, mybir=mybir, tile=tile)

    nc.compile()
    return nc


def _kernel_body(tc, xT, ci, cig, wq, sT, snat, znat, eoh, bias, y, *,
                 bass, mybir, tile):
    from contextlib import ExitStack

    nc = tc.nc
    f32 = mybir.dt.float32
    bf16 = mybir.dt.bfloat16
    i32 = mybir.dt.int32
    i8 = mybir.dt.int8
    Alu = mybir.AluOpType
    Act = mybir.ActivationFunctionType

    with ExitStack() as ctx:
        const = ctx.enter_context(tc.tile_pool(name="const", bufs=1))
        xstage = ctx.enter_context(tc.tile_pool(name="xstage", bufs=1))
        wqpool = ctx.enter_context(tc.tile_pool(name="wq", bufs=40))
        wnpool = ctx.enter_context(tc.tile_pool(name="wn", bufs=6))
        wtpool = ctx.enter_context(tc.tile_pool(name="wt", bufs=3))
        ypool = ctx.enter_context(tc.tile_pool(name="yout", bufs=2))
        psum = ctx.enter_context(tc.tile_pool(name="psum", bufs=1, space="PSUM"))

        # --- constants ---
        old_gather = bool(os.environ.get("KERNEL_OLD_GATHER"))
        if old_gather:
            ci_sb = const.tile([128, NT], i32, tag="ci")
            nc.sync.dma_start(out=ci_sb, in_=ci)
        else:
            cig_sb = const.tile([128, IN // 16], mybir.dt.int16, tag="cig")
            nc.scalar.dma_start(out=cig_sb, in_=cig)

        # scales, o-partition-major flat [128, OB*NT]: broadcast operand
        # of the q*s dequant multiply
        sT_sb = const.tile([128, OB * NT], f32, tag="sT")
        for ob in range(OB):
            nc.scalar.dma_start(out=sT_sb[:, ob * NT:(ob + 1) * NT], in_=sT[ob])

        # t-major scale/zero tables for the zero-point correction matmul:
        # nzs[t, o] = -(z * s), bf16
        snat_sb = const.tile([64, O_SLAB], f32, tag="snat")
        znat_sb = const.tile([64, O_SLAB], f32, tag="znat")
        nc.scalar.dma_start(out=snat_sb, in_=snat)
        nc.scalar.dma_start(out=znat_sb, in_=znat)
        nzs_f = const.tile([64, O_SLAB], f32, tag="nzsf")
        nc.vector.tensor_tensor(
            out=nzs_f[:, :], in0=znat_sb[:, :], in1=snat_sb[:, :], op=Alu.mult)
        nzs_b = const.tile([64, O_SLAB], bf16, tag="nzsb")
        nc.vector.tensor_scalar(
            out=nzs_b[:, :], in0=nzs_f[:, :],
            scalar1=-1.0, scalar2=None, op0=Alu.mult)

        # one-hot stationaries for the per-tile x column-sum (S1) matmuls
        eoh_sb = const.tile([128, NT * 64], bf16, tag="eoh")
        nc.scalar.dma_start(out=eoh_sb, in_=eoh)

        ones = const.tile([128, 128], bf16, tag="ones")
        nc.vector.memset(ones, 1.0)

        wbias = const.tile([128, O_SLAB], bf16, tag="wbias")
        nc.vector.memset(wbias, 0.0)
        bias_f = const.tile([1, O_SLAB], f32, tag="biasf")
        nc.scalar.dma_start(out=bias_f, in_=bias)
        nc.vector.tensor_copy(wbias[0:1, :], bias_f)

        # ---- x path: stage ALL of permuted x upfront (4.2 MB, 32KB/
        # partition). The gathers depend only on xT+indices, so the Pool
        # engine emits descriptors flat-out from t=0 instead of pacing the
        # per-chunk pipeline.
        # one tile per 1024-k block: writes to a single shared tile get
        # serialized by Tile with ~7.5us gpsimd DRAINs between gathers,
        # stretching the x feed to ~170us and starving the mid-game MMs
        xtiles = [xstage.tile([128, 8, B], bf16, tag=f"xall{g}",
                              name=f"xall{g}") for g in range(NT // 8)]
        if old_gather:
            for kt in range(NT):
                nc.gpsimd.indirect_dma_start(
                    out=xtiles[kt // 8][:, kt % 8, :],
                    out_offset=None,
                    in_=xT,
                    in_offset=bass.IndirectOffsetOnAxis(
                        ap=ci_sb[:, kt:kt + 1], axis=0),
                )
        else:
            for g in range(IN // 1024):
                # >=2048 rows per gather overflows the SWDGE descriptor
                # ring (crashes HW); 1024 is the safe max
                nc.gpsimd.dma_gather(
                    xtiles[g][:, :, :],
                    xT,
                    cig_sb[:, g * 64:(g + 1) * 64],
                    1024,
                    1024,
                    B,
                )

        # PSUM accumulation groups: [b-block][o-half]
        ps = [[psum.tile([128, 512], f32, tag=f"ps{bb}{oc}", name=f"ps{bb}{oc}")
               for oc in range(2)] for bb in range(2)]
        # per-k-tile x column sums S1[t, b], one long accumulation group
        s1ps = psum.tile([64, B], f32, tag="s1ps", name="s1ps")

        for ci_, (k_off, k_len) in enumerate(CHUNK_PLAN):
            tpc = k_len // 128
            # ---- W path: load int8, dequant, transpose to k-major ----
            # wt layout: [k-in-tile 128, half, t, ob-in-half, o-in-block 128]
            wt = wtpool.tile([128, 2, tpc, OB // 2, 128], bf16, tag="wt",
                             name=f"wt{ci_}")
            # per-half dequant staging [o-in-block, (t, obh, k)] so one xbar
            # transpose instruction covers many 128x128 blocks, landing at
            # wt[:, h, t, obh, :] directly
            wnh = [wnpool.tile([128, tpc, OB // 2, 128], bf16, tag="wn",
                               name=f"wn{ci_}h{h}") for h in range(2)]
            for ob in range(OB):
                wn, obh = wnh[ob // (OB // 2)], ob % (OB // 2)
                wq_sb = wqpool.tile([128, tpc, 128], i8, tag="wq",
                                    name=f"wq{ci_}o{ob}")
                nc.scalar.dma_start(
                    out=wq_sb,
                    in_=wq[ob * 128:(ob + 1) * 128,
                           k_off:k_off + k_len].rearrange(
                               "p (t k) -> p t k", t=tpc))
                # scale-only dequant q*s as ONE broadcast tensor_tensor per
                # (ob, chunk): the per-tile scale rides a stride-0 inner
                # axis, cutting 512 per-tile instructions (~270ns fixed
                # overhead each) to 40. The zero-point term is restored
                # exactly by the S1 correction matmuls below.
                sc0 = ob * NT + k_off // 128
                nc.vector.tensor_tensor(
                    out=wn[:, :, obh, :],
                    in0=wq_sb[:, :, :],
                    in1=sT_sb[:, sc0:sc0 + tpc, None].to_broadcast(
                        [128, tpc, 128]),
                    op=Alu.mult,
                )
            # both halves on the ACT ring (same-ring transposes are safe;
            # cross-ring split corrupted on HW) - oc=0 MMs can start after
            # the first half lands
            # both halves on the SAME ring (cross-ring split corrupted on
            # HW); Sync ring is otherwise transfer-free so the ~8.5us-per-
            # transpose queue occupancy does not block loads or dequant
            nc.sync.dma_start_transpose(
                wt[:, 0, :, :, :],
                wnh[0][:, :, :, :].rearrange("p a b c -> p (a b c)"))
            nc.sync.dma_start_transpose(
                wt[:, 1, :, :, :],
                wnh[1][:, :, :, :].rearrange("p a b c -> p (a b c)"))

            # ---- matmuls: accumulate y over this chunk's k-tiles ----
            # oc-major: the oc half only depends on its half-transpose.
            # The S1 matmuls (which depend only on xall) sit BETWEEN the
            # halves: they bridge the wait for the h1 transpose and keep
            # the PE HAM clock warm
            for oc in range(2):
                if oc == 1:
                    for t in range(tpc):
                        kt = k_off // 128 + t
                        nc.tensor.matmul(
                            s1ps[:, :],
                            eoh_sb[:, kt * 64:(kt + 1) * 64],
                            xtiles[kt // 8][:, kt % 8, :],
                            start=(kt == 0),
                            stop=(kt == NT - 1),
                        )
                for t in range(tpc):
                    kt = k_off // 128 + t
                    for bb in range(2):
                        nc.tensor.matmul(
                            ps[bb][oc][:, :],
                            xtiles[kt // 8][:, kt % 8,
                                            bb * 128:(bb + 1) * 128],
                            wt[:, oc, t, :, :],
                            start=(kt == 0),
                            stop=False,
                        )

        # ---- epilogue: zero-point correction + bias close each group ----
        # y -= S1 @ (z*s): lhsT = S1[t, b] bf16, rhs = -(z*s)[t, o] bf16,
        # contraction over the 64 k-tiles
        s1bf = const.tile([64, B], bf16, tag="s1bf")
        nc.scalar.copy(s1bf, s1ps[:, :])
        for oc in range(2):
            for bb in range(2):
                nc.tensor.matmul(
                    ps[bb][oc][:, :],
                    s1bf[:, bb * 128:(bb + 1) * 128],
                    nzs_b[:, oc * 512:(oc + 1) * 512],
                    start=False,
                    stop=False,
                )
                nc.tensor.matmul(
                    ps[bb][oc][:, :],
                    ones,
                    wbias[:, oc * 512:(oc + 1) * 512],
                    start=False,
                    stop=True,
                )
                ysb = ypool.tile([128, 512], f32, tag="ysb")
                nc.vector.tensor_copy(ysb, ps[bb][oc][:, :])
                nc.sync.dma_start(
                    out=y[bb * 128:(bb + 1) * 128, oc * 512:(oc + 1) * 512],
                    in_=ysb,
                )


def get_program():
    global _PROGRAM
    if _PROGRAM is None:
        _PROGRAM = _build_program()
    return _PROGRAM


def make_in_maps(x, W_high_q, W_low_q, scales_high, zeros_high,
                 scales_low, zeros_low, bias, col_indices):
    """Host-side sharding / layout prep. Returns per-core input dicts."""
    import concourse.mybir as mybir
    bf16 = mybir.dt.np(mybir.dt.bfloat16)

    x = np.asarray(x)
    xT = np.ascontiguousarray(x.T.astype(np.float32, copy=False).astype(bf16))
    ci = np.ascontiguousarray(
        np.asarray(col_indices).astype(np.int32, copy=False).reshape(NT, 128).T
    )  # [128, NT]; ci[p, t] = col_indices[t*128 + p]
    # dma_gather index layout: int16, wrapped into 16 partitions
    # (idx j at [j%16, j//16]), replicated to all 128 partitions
    cig = np.tile(
        np.asarray(col_indices).astype(np.int16, copy=False).reshape(IN // 16, 16).T,
        (8, 1),
    )  # [128, IN//16]

    s_all = np.concatenate(
        [np.asarray(scales_high, dtype=np.float32),
         np.asarray(scales_low, dtype=np.float32)], axis=0)   # [NT, OUT]
    z_all = np.concatenate(
        [np.asarray(zeros_high, dtype=np.float32),
         np.asarray(zeros_low, dtype=np.float32)], axis=0)    # [NT, OUT]
    sT_full = np.ascontiguousarray(s_all.T)                   # [OUT, NT]
    zT_full = np.ascontiguousarray(z_all.T)                   # [OUT, NT]

    # codes are 0..63: pack losslessly to int8, one merged [OUT, IN] tensor
    wq_full = np.concatenate(
        [np.asarray(W_high_q), np.asarray(W_low_q)], axis=1).astype(np.int8)
    bias = np.asarray(bias, dtype=np.float32)

    # one-hot stationaries for the S1 (per-tile x column sum) matmuls:
    # eoh[:, kt*64 + m] = 1 iff m == kt
    eoh = np.zeros((128, NT, 64), dtype=np.float32)
    for kt in range(NT):
        eoh[:, kt, kt] = 1.0
    eoh = np.ascontiguousarray(eoh.reshape(128, NT * 64).astype(bf16))

    in_maps = []
    for c in range(N_CORES):
        sl = slice(c * O_SLAB, (c + 1) * O_SLAB)
        in_maps.append({
            "xT": xT,
            "ci": ci,
            "cig": np.ascontiguousarray(cig),
            "wq": np.ascontiguousarray(wq_full[sl]),
            "sT": np.ascontiguousarray(sT_full[sl].reshape(OB, 128, NT)),
            "snat": np.ascontiguousarray(s_all[:, sl]),
            "znat": np.ascontiguousarray(z_all[:, sl]),
            "eoh": eoh,
            "bias": np.ascontiguousarray(bias[sl].reshape(1, O_SLAB)),
        })
    return in_maps


def run_on_device(in_maps):
    from concourse.bass_utils import run_bass_kernel_spmd
    nc = get_program()
    res = run_bass_kernel_spmd(nc, in_maps, list(range(N_CORES)))
    out = np.concatenate(
        [res.results[c]["y"] for c in range(N_CORES)], axis=1)
    return np.ascontiguousarray(out.astype(np.float32, copy=False))


def kernel(x, W_high_q, W_low_q, scales_high, zeros_high,
           scales_low, zeros_low, bias, col_indices):
    in_maps = make_in_maps(x, W_high_q, W_low_q, scales_high, zeros_high,
                           scales_low, zeros_low, bias, col_indices)
    return run_on_device(in_maps)


# ---------------------------------------------------------------------------
# Benchmark path (test.py only): inputs parked on-device, jit built once,
# dispatches pipelined so the axon-tunnel round trip amortizes away.
# ---------------------------------------------------------------------------

class DeviceRunner:
    def __init__(self, in_maps, nc=None):
        import jax
        import numpy as _np
        from jax.experimental.shard_map import shard_map
        from jax.sharding import Mesh, NamedSharding, PartitionSpec
        import concourse.mybir as mybir
        from concourse.bass2jax import (
            _bass_exec_p, install_neuronx_cc_hook, partition_id_tensor)

        install_neuronx_cc_hook()
        if nc is None:
            nc = get_program()
        partition_name = (nc.partition_id_tensor.name
                          if nc.partition_id_tensor else None)

        in_names, out_names, out_avals, zero_outs = [], [], [], []
        for alloc in nc.m.functions[0].allocations:
            if not isinstance(alloc, mybir.MemoryLocationSet):
                continue
            name = alloc.memorylocations[0].name
            if alloc.kind == "ExternalInput":
                if name != partition_name:
                    in_names.append(name)
            elif alloc.kind == "ExternalOutput":
                shape = tuple(alloc.tensor_shape)
                dtype = mybir.dt.np(alloc.dtype)
                out_names.append(name)
                out_avals.append(jax.core.ShapedArray(shape, dtype))
                zero_outs.append(_np.zeros(shape, dtype))
        n_params = len(in_names)
        all_in_names = list(in_names) + list(out_names)
        if partition_name is not None:
            all_in_names.append(partition_name)

        def _body(*args):
            operands = list(args)
            if partition_name is not None:
                operands.append(partition_id_tensor())
            return tuple(_bass_exec_p.bind(
                *operands,
                out_avals=tuple(out_avals),
                in_names=tuple(all_in_names),
                out_names=tuple(out_names),
                lowering_input_output_aliases=(),
                sim_require_finite=True,
                sim_require_nnan=True,
                nc=nc,
            ))

        devices = jax.devices()[:N_CORES]
        mesh = Mesh(_np.asarray(devices), ("core",))
        spec = PartitionSpec("core")
        nin = n_params + len(zero_outs)
        self.fn = jax.jit(
            shard_map(_body, mesh=mesh,
                      in_specs=(spec,) * nin,
                      out_specs=(spec,) * len(out_names),
                      check_rep=False),
            keep_unused=True,
        )
        sharding = NamedSharding(mesh, spec)
        concat_in = [
            _np.concatenate([in_maps[c][k] for c in range(N_CORES)], axis=0)
            for k in in_names
        ]
        concat_zeros = [
            _np.zeros((N_CORES * z.shape[0], *z.shape[1:]), z.dtype)
            for z in zero_outs
        ]
        self.args = [jax.device_put(a, sharding)
                     for a in concat_in + concat_zeros]
        self.out_names = out_names
        self.out_avals = out_avals
        self._jax = jax

    def run(self):
        return self.fn(*self.args)

    def fetch(self, outs):
        import numpy as _np
        y = _np.asarray(outs[self.out_names.index("y")])
        y = y.reshape(N_CORES, B, O_SLAB)
        return _np.concatenate(list(y), axis=1)

    def bench(self, iters=20):
        import time
        jax = self._jax
        # warm
        outs = self.run()
        jax.block_until_ready(outs)
        t0 = time.perf_counter()
        last = None
        for _ in range(iters):
            last = self.run()
        jax.block_until_ready(last)
        dt = (time.perf_counter() - t0) / iters
        return dt, self.fetch(last)
